# revision 17
# baseline (speedup 1.0000x reference)
"""Trainium2 Bass kernel for nn_ATSA_56384330662502 (topk_masking), v3.

Math (validated against the reference in fp-sim, rel err ~1.2e-3, tol 2e-2):
  a_k selection needs only the top-a_k tokens by imp (pooled_sum telescopes to
  total - sum_selected).  The screen therefore only has to put the true top
  token(s) into a candidate set; an exact fp16 rescore picks the winner.

  screen: 128-wide hidden subset of p_w1/p_w2 (126 units with largest
  |w2_h|*||W1_h|| + 2 slots (+v,-v) that synthesize the rank-1 linear
  correction 0.5*sum_dropped w2_h W1_h of the dropped units; relu(x)-relu(-x)
  = x).  fp8-e4m3 DoubleRow matmuls (weights *16).  Scores are produced per
  (sample, 512-token quarter) in a [16, 512] layout so ONE DVE max/max_index
  call yields top-8 per quarter = 32 candidates/sample (validated: true top-1
  has worst in-quarter screen rank 2).
  rescore: gather the 32 fp32 rows, exact fp16 full-H rescore ranks them
  (validated exact for every sample).
  totals: fp8 sums split across DVE (tensor_tensor_reduce, 2 elem/cyc),
  ACT (accum-copy) and GPSIMD (pair-add) so no engine is the bottleneck.
  pooled = (total - sum_sel)/(N - a_k); agg = (sum_ref + pooled)/(a_k+1)
  out = mlp2(agg, f_*)  (f16 weights)

Sharding: data-parallel over batch, 8 samples/core.  tok8 ships transposed
e4m3 packed so each sample's DMA is one contiguous 16KB descriptor per
partition; tok_nat fp32 natural is only touched by the candidate gather.
"""
import os
import numpy as np
import ml_dtypes

import concourse.bass as bass
import concourse.mybir as mybir
import concourse.bacc as bacc
import concourse.tile as tile
from concourse.bass_utils import run_bass_kernel_spmd
from concourse.masks import make_identity

F32 = mybir.dt.float32
F16 = mybir.dt.float16
FP8 = mybir.dt.float8e4
U32 = mybir.dt.uint32
I32 = mybir.dt.int32
AF = mybir.ActivationFunctionType
OP = mybir.AluOpType
AX = mybir.AxisListType
DR = mybir.MatmulPerfMode.DoubleRow

B, N, C, H = 64, 2048, 1024, 512
NCORES = 8
BS = B // NCORES            # 8 samples per core
R = BS * N                  # 16384 token rows per core
KC = C // 128               # 8 contraction chunks
JP = KC // 2                # 4 DoubleRow chunk-pairs
M4 = H // 128               # 4 chunks of H
NQ = 4                      # quarters per sample
QW = N // NQ                # 512
HSUB = 128                  # screen hidden width (126 subset + 2 comp)
KD = 8                      # top-8 per quarter
CPS = NQ * KD               # 32 candidates per sample
CPB = 4 * CPS               # 128 candidates per 4-sample batch
H2 = H // 2                 # 256
KH2 = H2 // 128             # 2

# totals chunk split: chunks 0..NDV-1 -> DVE plain reduce, next NAC -> ACT
# accum-copy, rest -> GPSIMD pair-add + DVE fp16 finish
NDV = 2
NAC = 3

_last_results = None


def _floor_pos(nc, pool, src_ap, tag):
    """floor(x) for x >= 0; fp32->int32 cast is round-to-nearest-even, so
    floor(x) == rne(x - 0.5) (x never an exact integer here)."""
    ti = pool.tile([1, BS], I32, tag=tag + "_i", name=tag + "_i")
    tf = pool.tile([1, BS], F32, tag=tag + "_f", name=tag + "_f")
    th = pool.tile([1, BS], F32, tag=tag + "_h", name=tag + "_h")
    nc.vector.tensor_scalar(th[:], src_ap, 0.5, None, op0=OP.subtract)
    nc.vector.tensor_copy(ti[:], th[:])
    nc.vector.tensor_copy(tf[:], ti[:])
    return tf


def build_program():
    nc = bacc.Bacc("TRN2", target_bir_lowering=False, debug=False,
                   num_devices=NCORES)

    def din(name, shape, dt=F32):
        return nc.dram_tensor(name, list(shape), dt, kind="ExternalInput").ap()

    tok8p = din("tok8p", [128, BS * KC * N], FP8)   # packed transposed shard
    tok_nat = din("tok_nat", [R, C])                # natural shard (gather)
    w1dr = din("w1dr", [C, HSUB], FP8)              # screen W1' * 16, e4m3
    w2sel = din("w2sel", [128, 256], F16)           # screen w2' sl-masked
    pw1f = din("pw1f", [C, H], F16)                 # p_w1 (rescore)
    w2f64 = din("w2f64", [128, 4 * M4 * 4])         # p_w2 sl-masked (rescore)
    enc_w = din("enc_w", [C, H], F16)
    a_w1 = din("a_w1", [H, H2], F16); a_w2 = din("a_w2", [H2, 1], F16)
    k_w1 = din("k_w1", [H, H2], F16); k_w2 = din("k_w2", [H2, 1], F16)
    r_w1 = din("r_w1", [C, H], F16); r_w2 = din("r_w2", [H, C], F16)
    f_w1 = din("f_w1", [C, H], F16); f_w2 = din("f_w2", [H, C], F16)
    a_b2 = din("a_b2", [1, 1]); k_b2 = din("k_b2", [1, 1])
    consts = din("consts", [128, 40])               # bundled biases
    rowbase = {b: din(f"rowbase{b}", [16, 1]) for b in range(2)}

    out_t = nc.dram_tensor("out_t", [C, BS], F32, kind="ExternalOutput").ap()

    with tile.TileContext(nc) as tc:
        with tc.tile_pool(name="wp", bufs=1) as wp, \
             tc.tile_pool(name="xb", bufs=2) as xbp, \
             tc.tile_pool(name="rh", bufs=2) as rhp, \
             tc.tile_pool(name="jk", bufs=2) as jkp, \
             tc.tile_pool(name="gb", bufs=3) as gbp, \
             tc.tile_pool(name="sc", bufs=2) as scp, \
             tc.tile_pool(name="ps", bufs=2, space="PSUM") as php, \
             tc.tile_pool(name="pt", bufs=2, space="PSUM") as ptp, \
             tc.tile_pool(name="pi", bufs=1, space="PSUM") as pip, \
             tc.tile_pool(name="dp", bufs=1, space="DRAM") as dp:

            # ---- persistent fp8 weights + consts (needed before sample 0) ----
            w1sb = wp.tile([128, KC, HSUB], FP8, tag="w1sb", name="w1sb")
            nc.sync.dma_start(w1sb[:], w1dr.rearrange("(j p) h -> p j h", p=128))
            w2sb = wp.tile([128, 256], F16, tag="w2sb", name="w2sb")
            nc.sync.dma_start(w2sb[:], w2sel)
            cst = wp.tile([128, 40], F32, tag="cst", name="cst")
            nc.sync.dma_start(cst[:], consts)
            pb1s = cst[:, 0:1]    # screen relu bias (*16)
            encb = cst[:, 4:8]; ab1 = cst[:, 8:10]; kb1 = cst[:, 10:12]
            rb1 = cst[:, 12:16]; rb2 = cst[:, 16:24]
            fb1 = cst[:, 24:28]; fb2 = cst[:, 28:36]
            pb1 = cst[:, 36:40]   # p_b1 (rescore relu)
            rwb = {}
            for b in range(2):
                rwb[b] = wp.tile([16, 1], F32, tag=f"rwb{b}", name=f"rwb{b}")
                nc.sync.dma_start(rwb[b][:], rowbase[b])

            tot3 = wp.tile([128, KC, BS], F32, tag="tot3", name="tot3")
            impq = {b: wp.tile([16, QW], F32, tag=f"impq{b}", name=f"impq{b}")
                    for b in range(2)}
            mx = {b: wp.tile([16, 8], F32, tag=f"mx{b}", name=f"mx{b}")
                  for b in range(2)}
            ixf = {b: wp.tile([16, KD], U32, tag=f"ixf{b}", name=f"ixf{b}")
                   for b in range(2)}
            ixg = {b: wp.tile([16, KD], F32, tag=f"ixg{b}", name=f"ixg{b}")
                   for b in range(2)}
            ixi = {b: wp.tile([16, KD], I32, tag=f"ixi{b}", name=f"ixi{b}")
                   for b in range(2)}
            gidx = {b: wp.tile([CPB, 1], I32, tag=f"gidx{b}", name=f"gidx{b}")
                    for b in range(2)}
            gath = {b: wp.tile([CPB, C], F32, tag=f"gath{b}", name=f"gath{b}")
                    for b in range(2)}
            gathT = {b: wp.tile([128, KC * CPB], F32, tag=f"gT{b}",
                                name=f"gT{b}") for b in range(2)}
            gathT16 = {b: wp.tile([128, KC * CPB], F16, tag=f"gT16{b}",
                                  name=f"gT16{b}") for b in range(2)}
            rhr = {b: wp.tile([128, M4 * CPB], F32, tag=f"rhr{b}",
                              name=f"rhr{b}") for b in range(2)}
            impr = {b: wp.tile([4, CPS], F32, tag=f"impr{b}", name=f"impr{b}")
                    for b in range(2)}
            akf4 = {b: wp.tile([4, 1], F32, tag=f"akf4{b}", name=f"akf4{b}")
                    for b in range(2)}
            mflat = {b: wp.tile([1, CPB], F32, tag=f"mf{b}", name=f"mf{b}")
                     for b in range(2)}
            scrm = dp.tile([CPB, 2], F32, tag="scrm", name="scrm")
            bcm = {b: wp.tile([128, CPB], F32, tag=f"bcm{b}", name=f"bcm{b}")
                   for b in range(2)}
            rr = {b: wp.tile([128, M4 * CPB], F16, tag=f"rr{b}",
                             name=f"rr{b}") for b in range(2)}
            rrs = {b: wp.tile([128, M4 * 4], F16, tag=f"rrs{b}",
                              name=f"rrs{b}") for b in range(2)}
            rrsF = wp.tile([128, M4 * 4], F32, tag="rrsF", name="rrsF")
            srefA = wp.tile([128, KC * BS], F32, tag="srefA", name="srefA")
            sselA = wp.tile([128, KC * BS], F32, tag="sselA", name="sselA")
            scratch = dp.tile([CPB, 2], I32, tag="scratch", name="scratch")

            def tail_weights():
                def load_mat(dram, kdim, mwidth, dt, name):
                    t = wp.tile([128, kdim * mwidth], dt, tag=name, name=name)
                    nc.sync.dma_start(
                        t[:].rearrange("p (k m) -> p k m", k=kdim),
                        dram.rearrange("(k p) m -> p k m", p=128))
                    return t
                pw1sb = load_mat(pw1f, KC, H, F16, "pw1sb")
                encw = load_mat(enc_w, KC, H, F16, "encw")
                aw1 = load_mat(a_w1, M4, H2, F16, "aw1")
                aw2 = load_mat(a_w2, KH2, 1, F16, "aw2")
                kw1 = load_mat(k_w1, M4, H2, F16, "kw1")
                kw2 = load_mat(k_w2, KH2, 1, F16, "kw2")
                rw1 = load_mat(r_w1, KC, H, F16, "rw1")
                rw2 = load_mat(r_w2, M4, C, F16, "rw2")
                fw1 = load_mat(f_w1, KC, H, F16, "fw1")
                fw2 = load_mat(f_w2, M4, C, F16, "fw2")
                w2fsb = wp.tile([128, 4 * M4 * 4], F32, tag="w2fsb",
                                name="w2fsb")
                nc.sync.dma_start(w2fsb[:], w2f64)
                ab2t = wp.tile([1, 1], F32, tag="ab2", name="ab2")
                nc.sync.dma_start(ab2t[:], a_b2)
                kb2t = wp.tile([1, 1], F32, tag="kb2", name="kb2")
                nc.sync.dma_start(kb2t[:], k_b2)
                ident = wp.tile([128, 128], F32, tag="ident", name="ident")
                make_identity(nc, ident[:])
                ones1 = wp.tile([1, 128], F32, tag="ones1", name="ones1")
                nc.gpsimd.memset(ones1[:], 1.0)
                return (pw1sb, encw, aw1, aw2, kw1, kw2, rw1, rw2, fw1, fw2,
                        w2fsb, ab2t, kb2t, ident, ones1)

            # ============== tail stages (per 4-sample batch) ==============
            def top8_batch(b):
                nc.scalar.activation(impq[b][:], SC[b][:], AF.Copy)
                nc.vector.max(mx[b][:], impq[b][:])
                nc.vector.max_index(ixf[b][:], mx[b][:], impq[b][:])
                nc.vector.tensor_copy(ixg[b][:], ixf[b][:])
                nc.vector.tensor_scalar(ixg[b][:], ixg[b][:],
                                        rwb[b][:], None, op0=OP.add)
                nc.vector.tensor_copy(ixi[b][:], ixg[b][:])
                # bounce [16, 8] -> [128, 1] through a DRAM tile (dep-tracked)
                nc.sync.dma_start(
                    scratch[:, b:b + 1]
                    .rearrange("(r c) x -> r (c x)", c=KD),
                    ixi[b][:])
                nc.sync.dma_start(gidx[b][:], scratch[:, b:b + 1])
                nc.gpsimd.indirect_dma_start(
                    out=gath[b][:], out_offset=None, in_=tok_nat,
                    in_offset=bass.IndirectOffsetOnAxis(ap=gidx[b][:, 0:1],
                                                        axis=0))

            def transpose_batch(b, ident):
                # gath [CPB, C] fp32 -> gathT [128, (cc, cand)] + f16 copy
                for g in range(2):          # two groups of 4 chunks
                    pt = ptp.tile([128, 512], F32, tag="ph", name="pt")
                    for cc in range(4 * g, 4 * g + 4):
                        nc.tensor.transpose(
                            pt[:, 128 * (cc - 4 * g):128 * (cc - 4 * g) + 128],
                            gath[b][:, 128 * cc:128 * (cc + 1)], ident[:])
                    lo = 512 * g
                    nc.scalar.activation(gathT[b][:, lo:lo + 512], pt[:],
                                         AF.Copy)
                    nc.vector.tensor_copy(gathT16[b][:, lo:lo + 512],
                                          gathT[b][:, lo:lo + 512])

            def rescore_batch(b, pw1sb, w2fsb, ones1, bcv, ident):
                # exact fp16 full-H rescore of the CPB gathered candidates
                pr = ptp.tile([128, M4 * 128], F32, tag="ph", name="pr")
                for m in range(M4):
                    for j in range(KC):
                        nc.tensor.matmul(
                            pr[:, 128 * m:128 * (m + 1)],
                            pw1sb[:, H * j + 128 * m:H * j + 128 * (m + 1)],
                            gathT16[b][:, 128 * j:128 * (j + 1)],
                            start=(j == 0), stop=(j == KC - 1))
                for m in range(M4):
                    nc.scalar.activation(
                        rhr[b][:, 128 * m:128 * (m + 1)],
                        pr[:, 128 * m:128 * (m + 1)],
                        AF.Relu, bias=pb1[:, m:m + 1])
                # stage 2 into [4 samples, CPS] layout via sl-masked w2
                pR = ptp.tile([4, CPS], F32, tag="ph", name="pR")
                for m in range(M4):
                    for sl in range(4):
                        nc.tensor.matmul(
                            pR[:], w2fsb[:, 4 * (4 * m + sl):
                                         4 * (4 * m + sl) + 4],
                            rhr[b][:, 128 * m + CPS * sl:
                                   128 * m + CPS * sl + CPS],
                            start=(m == 0 and sl == 0),
                            stop=(m == M4 - 1 and sl == 3))
                nc.scalar.activation(impr[b][:], pR[:], AF.Copy)
                # rank candidates within their sample, mask = rank < a_k
                cmp4 = scp.tile([4, CPS * CPS], F32, tag="cmp4",
                                name="cmp4", bufs=1)
                vA = impr[b][:].rearrange("p (c o) -> p c o", o=1) \
                    .to_broadcast([4, CPS, CPS])
                vB = impr[b][:].rearrange("p (o c) -> p o c", o=1) \
                    .to_broadcast([4, CPS, CPS])
                nc.vector.tensor_tensor(
                    cmp4[:].rearrange("p (c o) -> p c o", o=CPS),
                    vB, vA, op=OP.is_gt)
                rank = scp.tile([4, CPS], F32, tag="rank", name="rank",
                                bufs=1)
                nc.vector.tensor_reduce(
                    rank[:], cmp4[:].rearrange("p (c o) -> p c o", o=CPS),
                    axis=AX.X, op=OP.add)
                # a_k per sample as a [4, 1] column via PE transpose
                pak = ptp.tile([4, 1], F32, tag="ph", name="pak")
                nc.tensor.transpose(pak[:],
                                    bcv[:, 2 * BS + 4 * b:2 * BS + 4 * b + 4],
                                    ident[0:1, 0:1])
                nc.scalar.activation(akf4[b][:], pak[:], AF.Copy)
                mask1 = scp.tile([4, CPS], F32, tag="mask1", name="mask1",
                                 bufs=1)
                nc.vector.tensor_scalar(mask1[:], rank[:], akf4[b][:, 0:1],
                                        None, op0=OP.is_lt)
                # bounce [4, CPS] -> [1, CPB], broadcast to 128 rows via PE
                nc.sync.dma_start(
                    scrm[:, b:b + 1].rearrange("(s c) x -> s (c x)", c=CPS),
                    mask1[:])
                nc.sync.dma_start(
                    mflat[b][:],
                    scrm[:, b:b + 1].rearrange("(a c) x -> a (c x)", c=CPB))
                pbm = ptp.tile([128, CPB], F32, tag="ph", name="pbm")
                nc.tensor.matmul(pbm[:], ones1[:], mflat[b][:], start=True,
                                 stop=True)
                nc.scalar.activation(bcm[b][:], pbm[:], AF.Copy)

            def refine_batch(b, rw1, rw2):
                # mlp2(cand, r_*) for all CPB candidates, mask-summed / sample
                prf = ptp.tile([128, M4 * 128], F32, tag="ph", name="prf")
                for m in range(M4):
                    for j in range(KC):
                        nc.tensor.matmul(
                            prf[:, 128 * m:128 * (m + 1)],
                            rw1[:, H * j + 128 * m:H * j + 128 * (m + 1)],
                            gathT16[b][:, 128 * j:128 * (j + 1)],
                            start=(j == 0), stop=(j == KC - 1))
                for m in range(M4):
                    nc.scalar.activation(
                        rr[b][:, 128 * m:128 * (m + 1)],
                        prf[:, 128 * m:128 * (m + 1)],
                        AF.Relu, bias=rb1[:, m:m + 1])
                # mask + per-sample presum over candidates (32 contiguous)
                rrm = scp.tile([128, M4 * CPB], F16, tag="rrm", name="rrm",
                               bufs=1)
                nc.vector.tensor_tensor(
                    rrm[:].rearrange("p (m c) -> p m c", m=M4),
                    rr[b][:].rearrange("p (m c) -> p m c", m=M4),
                    bcm[b][:].rearrange("p (o c) -> p o c", o=1)
                    .to_broadcast([128, M4, CPB]), op=OP.mult)
                nc.vector.tensor_reduce(
                    rrsF[:],
                    rrm[:].rearrange("p (ms c) -> p ms c", c=CPS),
                    axis=AX.X, op=OP.add)
                nc.vector.tensor_copy(rrs[b][:], rrsF[:])
                # stage 2: out[c-chunk, sample] directly
                prg = ptp.tile([128, KC * 4], F32, tag="ph", name="prg")
                for cc in range(KC):
                    for m in range(M4):
                        nc.tensor.matmul(
                            prg[:, 4 * cc:4 * cc + 4],
                            rw2[:, C * m + 128 * cc:C * m + 128 * (cc + 1)],
                            rrs[b][:, 4 * m:4 * m + 4],
                            start=(m == 0), stop=(m == M4 - 1))
                nc.scalar.activation(
                    srefA[:].rearrange("p (k s) -> p k s", k=KC)
                    [:, :, 4 * b:4 * b + 4],
                    prg[:].rearrange("p (k s) -> p k s", k=KC), AF.Copy)

            def selsum_batch(b):
                # sum of selected raw rows per sample (fp32, from gathT)
                selm = scp.tile([128, KC * CPB], F32, tag="selm", name="selm",
                                bufs=1)
                nc.vector.tensor_tensor(
                    selm[:].rearrange("p (k c) -> p k c", k=KC),
                    gathT[b][:].rearrange("p (k c) -> p k c", k=KC),
                    bcm[b][:].rearrange("p (o c) -> p o c", o=1)
                    .to_broadcast([128, KC, CPB]), op=OP.mult)
                nc.vector.tensor_reduce(
                    sselA[:].rearrange("p (k s) -> p k s", k=KC)
                    [:, :, 4 * b:4 * b + 4],
                    selm[:].rearrange("p (k s c) -> p k s c", k=KC, c=CPS),
                    axis=AX.X, op=OP.add)

            # ---- router (per 4-sample batch, needs tot3 cols done) ----
            def router_batch(b, encw, aw1, aw2, kw1, kw2, ab2t, kb2t,
                             featT16, bcv, meanT16):
                s0, s1 = 4 * b, 4 * b + 4
                nc.vector.tensor_scalar(
                    meanT16[:, :, s0:s1], tot3[:, :, s0:s1], 1.0 / N, None,
                    op0=OP.mult)
                for m in range(M4):
                    pf = ptp.tile([128, 4], F32, tag="ph", name="pf")
                    for j in range(KC):
                        nc.tensor.matmul(pf[:], encw[:, H * j + 128 * m:
                                                     H * j + 128 * (m + 1)],
                                         meanT16[:, j, s0:s1], start=(j == 0),
                                         stop=(j == KC - 1))
                    nc.scalar.activation(featT16[m][:, s0:s1], pf[:], AF.Relu,
                                         bias=encb[:, m:m + 1])

                def head(w1t, b1c, w2t, b2t, kind, o):
                    h1 = {}
                    for mh in range(KH2):
                        p1 = ptp.tile([128, 4], F32, tag="ph", name="p1")
                        for k in range(M4):
                            nc.tensor.matmul(
                                p1[:], w1t[:, H2 * k + 128 * mh:
                                           H2 * k + 128 * (mh + 1)],
                                featT16[k][:, s0:s1], start=(k == 0),
                                stop=(k == M4 - 1))
                        t1 = scp.tile([128, 4], F16, tag=f"hh{o}{mh}{b}",
                                      name=f"hh{o}{mh}{b}", bufs=1)
                        nc.scalar.activation(t1[:], p1[:], AF.Relu,
                                             bias=b1c[:, mh:mh + 1])
                        h1[mh] = t1
                    p2 = ptp.tile([1, 4], F32, tag="ph", name=f"p2{o}")
                    for k in range(KH2):
                        nc.tensor.matmul(p2[:], w2t[:, k:k + 1], h1[k][:],
                                         start=(k == 0), stop=(k == KH2 - 1))
                    dst = alpha if kind == "sigmoid" else kraw
                    if kind == "sigmoid":
                        nc.scalar.activation(dst[:, s0:s1], p2[:], AF.Sigmoid,
                                             bias=b2t[:])
                    else:  # softplus = Ln(1 + Exp(x))
                        e = scp.tile([1, 4], F32, tag=f"se{b}", name=f"se{b}",
                                     bufs=1)
                        nc.scalar.activation(e[:], p2[:], AF.Exp, bias=b2t[:])
                        nc.vector.tensor_scalar(e[:], e[:], 1.0, None,
                                                op0=OP.add)
                        nc.scalar.activation(dst[:, s0:s1], e[:], AF.Ln)
                head(aw1, ab1, aw2, ab2t, "sigmoid", "a")
                head(kw1, kb1, kw2, kb2t, "softplus", "k")
                # k = clip(round(kraw),1,20); a_k = min(max(1,floor(alpha*k)),k)
                kr2 = scp.tile([1, BS], F32, tag="kr2", name="kr2", bufs=1)
                nc.vector.tensor_scalar(kr2[:, s0:s1], kraw[:, s0:s1], 0.5,
                                        None, op0=OP.add)
                ti = scp.tile([1, BS], I32, tag="kfi", name="kfi", bufs=1)
                tf = scp.tile([1, BS], F32, tag="kff", name="kff", bufs=1)
                nc.vector.tensor_scalar(tf[:, s0:s1], kr2[:, s0:s1], 0.5,
                                        None, op0=OP.subtract)
                nc.vector.tensor_copy(ti[:, s0:s1], tf[:, s0:s1])
                nc.vector.tensor_copy(kf[:, s0:s1], ti[:, s0:s1])
                nc.vector.tensor_scalar(kf[:, s0:s1], kf[:, s0:s1], 1.0, 20.0,
                                        op0=OP.max, op1=OP.min)
                ak0 = scp.tile([1, BS], F32, tag="ak0", name="ak0", bufs=1)
                nc.vector.tensor_tensor(ak0[:, s0:s1], alpha[:, s0:s1],
                                        kf[:, s0:s1], op=OP.mult)
                ti2 = scp.tile([1, BS], I32, tag="aki", name="aki", bufs=1)
                tf2 = scp.tile([1, BS], F32, tag="akh", name="akh", bufs=1)
                nc.vector.tensor_scalar(tf2[:, s0:s1], ak0[:, s0:s1], 0.5,
                                        None, op0=OP.subtract)
                nc.vector.tensor_copy(ti2[:, s0:s1], tf2[:, s0:s1])
                nc.vector.tensor_copy(akf[:, s0:s1], ti2[:, s0:s1])
                nc.vector.tensor_scalar_max(akf[:, s0:s1], akf[:, s0:s1], 1.0)
                nc.vector.tensor_tensor(akf[:, s0:s1], akf[:, s0:s1],
                                        kf[:, s0:s1], op=OP.min)
                # bcv: [inv1 | inv2 | a_k]
                cnt = scp.tile([1, BS], F32, tag="cnt", name="cnt", bufs=1)
                nc.vector.tensor_scalar(cnt[:, s0:s1], akf[:, s0:s1], -1.0,
                                        float(N), op0=OP.mult, op1=OP.add)
                nc.vector.reciprocal(bcv[:, s0:s1], cnt[:, s0:s1])
                ak1 = scp.tile([1, BS], F32, tag="ak1", name="ak1", bufs=1)
                nc.vector.tensor_scalar(ak1[:, s0:s1], akf[:, s0:s1], 1.0,
                                        None, op0=OP.add)
                nc.vector.reciprocal(bcv[:, BS + s0:BS + s1], ak1[:, s0:s1])
                nc.vector.tensor_copy(bcv[:, 2 * BS + s0:2 * BS + s1],
                                      akf[:, s0:s1])

            # shared router tiles
            alpha = wp.tile([1, BS], F32, tag="alpha", name="alpha")
            kraw = wp.tile([1, BS], F32, tag="kraw", name="kraw")
            kf = wp.tile([1, BS], F32, tag="kf", name="kf")
            akf = wp.tile([1, BS], F32, tag="akf", name="akf")
            bcv = wp.tile([1, 3 * BS], F32, tag="bcv", name="bcv")
            meanT16 = wp.tile([128, KC, BS], F16, tag="meanT", name="meanT")
            featT16 = {m: wp.tile([128, BS], F16, tag=f"featT{m}",
                                  name=f"featT{m}") for m in range(M4)}
            SC = {}

            # ================= main fp8 stream =================
            pending = []

            def flush_s2():
                while pending:
                    fs, fh, frh = pending.pop(0)
                    fb, fsl = divmod(fs, 4)
                    if fb not in SC:
                        SC[fb] = pip.tile([16, QW], F32, tag="SC",
                                          name=f"SC{fb}")
                    for qh in range(2):
                        q = 2 * fh + qh
                        r = 4 * fsl + q
                        nc.tensor.matmul(
                            SC[fb][:], w2sb[:, 16 * r:16 * r + 16],
                            frh[:, q, :],
                            start=(r == 0), stop=(r == 15))

            for s in range(BS):
                b, sl = divmod(s, 4)
                xb = xbp.tile([128, KC, N], FP8, tag="xb", name="xb")
                nc.sync.dma_start(
                    xb[:].rearrange("p j n -> p (j n)"),
                    tok8p[:, KC * N * s:KC * N * (s + 1)])
                # screen: two halves of 2 quarters each; stage-2 trails by
                # one half so the PE never waits on the relu copy-out
                rh16 = rhp.tile([128, NQ, QW], F16, tag="rh16", name="rh16")
                for hh in range(2):
                    ps1 = php.tile([128, 1024], F32, tag="ps1", name="ps1")
                    for jp in range(JP):
                        for qh in range(2):
                            q = 2 * hh + qh
                            nc.tensor.matmul(
                                ps1[:, 512 * qh:512 * qh + 512],
                                w1sb[:, 2 * jp:2 * jp + 2, :],
                                xb[:, 2 * jp:2 * jp + 2, QW * q:QW * (q + 1)],
                                start=(jp == 0), stop=(jp == JP - 1),
                                perf_mode=DR)
                    flush_s2()
                    dst = rh16[:, 2 * hh:2 * hh + 2, :] \
                        .rearrange("p a n -> p (a n)")
                    if hh == 0:
                        nc.scalar.activation(dst, ps1[:], AF.Relu,
                                             bias=pb1s[:])
                    else:
                        nc.vector.tensor_scalar(dst, ps1[:], pb1s[:], 0.0,
                                                op0=OP.add, op1=OP.max)
                    pending.append((s, hh, rh16))
                    if s == 4 and hh == 0:
                        # batch-0 scores complete (its last stage-2 flushed
                        # above); copy out before SC[1] reuses the bank
                        top8_batch(0)
                # totals: DVE {0,1} / ACT {2,3,4} / GP {5,6,7} + one DVE L2
                for j in range(2):
                    nc.vector.tensor_reduce(
                        tot3[:, j, s:s + 1], xb[:, j, :],
                        axis=AX.X, op=OP.add)
                junkA = scp.tile([128, N], FP8, tag="junkA", name="junkA",
                                 bufs=1)
                for j in range(2, 5):
                    nc.scalar.activation(
                        junkA[:], xb[:, j, :],
                        AF.Copy,
                        accum_out=tot3[:, j, s:s + 1])
                gbf = gbp.tile([128, 3, 1024], F16, tag="gbf", name="gbf")
                for i, j in enumerate(range(5, KC)):
                    nc.gpsimd.tensor_tensor(
                        gbf[:, i, :], xb[:, j, 0:1024], xb[:, j, 1024:2048],
                        op=OP.add)
                nc.vector.tensor_reduce(
                    tot3[:, 5:KC, s:s + 1], gbf[:],
                    axis=AX.X, op=OP.add)

                if s == 1:
                    (pw1sb, encw, aw1, aw2, kw1, kw2, rw1, rw2, fw1, fw2,
                     w2fsb, ab2t, kb2t, ident, ones1) = tail_weights()
                if s == 4:
                    router_batch(0, encw, aw1, aw2, kw1, kw2, ab2t, kb2t,
                                 featT16, bcv, meanT16)
                if s == 5:
                    transpose_batch(0, ident)
                if s == 6:
                    rescore_batch(0, pw1sb, w2fsb, ones1, bcv, ident)
                if s == 7:
                    refine_batch(0, rw1, rw2)
                    selsum_batch(0)
            flush_s2()

            # ================= tail =================
            top8_batch(1)
            router_batch(1, encw, aw1, aw2, kw1, kw2, ab2t, kb2t,
                         featT16, bcv, meanT16)
            transpose_batch(1, ident)
            rescore_batch(1, pw1sb, w2fsb, ones1, bcv, ident)
            refine_batch(1, rw1, rw2)
            selsum_batch(1)

            # srefA += r_b2 * a_k  (broadcast both)
            pbc = ptp.tile([128, 3 * BS], F32, tag="ph", name="pbc")
            nc.tensor.matmul(pbc[:], ones1[:], bcv[:], start=True, stop=True)
            bc = wp.tile([128, 3 * BS], F32, tag="bc", name="bc")
            nc.scalar.activation(bc[:], pbc[:], AF.Copy)
            rbt = scp.tile([128, KC * BS], F32, tag="rbt", name="rbt")
            nc.vector.tensor_tensor(
                rbt[:].rearrange("p (c s) -> p c s", c=KC),
                rb2[:].rearrange("p (c o) -> p c o", o=1)
                .to_broadcast([128, KC, BS]),
                bc[:, 2 * BS:3 * BS].rearrange("p (o s) -> p o s", o=1)
                .to_broadcast([128, KC, BS]), op=OP.mult)
            nc.vector.tensor_tensor(srefA[:], srefA[:], rbt[:], op=OP.add)

            # pooled + agg
            poodA = scp.tile([128, KC * BS], F32, tag="poodA", name="poodA")
            nc.vector.tensor_tensor(
                poodA[:], tot3[:].rearrange("p c s -> p (c s)"), sselA[:],
                op=OP.subtract)
            nc.vector.tensor_tensor(
                poodA[:].rearrange("p (c s) -> p c s", c=KC),
                poodA[:].rearrange("p (c s) -> p c s", c=KC),
                bc[:, 0:BS].rearrange("p (o s) -> p o s", o=1)
                .to_broadcast([128, KC, BS]), op=OP.mult)
            nc.vector.tensor_tensor(poodA[:], poodA[:], srefA[:], op=OP.add)
            aggA = wp.tile([128, KC * BS], F16, tag="aggA", name="aggA")
            nc.vector.tensor_tensor(
                aggA[:].rearrange("p (c s) -> p c s", c=KC),
                poodA[:].rearrange("p (c s) -> p c s", c=KC),
                bc[:, BS:2 * BS].rearrange("p (o s) -> p o s", o=1)
                .to_broadcast([128, KC, BS]), op=OP.mult)

            # ---- final MLP (f16) ----
            ff1 = {}
            for m in range(M4):
                pf1 = ptp.tile([128, BS], F32, tag="ph", name="pf1")
                for k in range(KC):
                    nc.tensor.matmul(pf1[:], fw1[:, H * k + 128 * m:
                                                 H * k + 128 * (m + 1)],
                                     aggA[:, BS * k:BS * (k + 1)],
                                     start=(k == 0), stop=(k == KC - 1))
                t = scp.tile([128, BS], F16, tag=f"ff1_{m}", name=f"ff1_{m}",
                             bufs=1)
                nc.scalar.activation(t[:], pf1[:], AF.Relu, bias=fb1[:, m:m + 1])
                ff1[m] = t
            for cc in range(KC):
                po = ptp.tile([128, BS], F32, tag="ph", name="po")
                for m in range(M4):
                    nc.tensor.matmul(po[:], fw2[:, C * m + 128 * cc:
                                                C * m + 128 * (cc + 1)],
                                     ff1[m][:], start=(m == 0),
                                     stop=(m == M4 - 1))
                oc = scp.tile([128, BS], F32, tag="oc", name="oc")
                nc.vector.tensor_scalar(oc[:], po[:], fb2[:, cc:cc + 1], None,
                                        op0=OP.add)
                nc.sync.dma_start(out_t[128 * cc:128 * (cc + 1), :], oc[:])

    nc.compile()
    return nc


def _install_ntff_shim():
    """This image's antenv lacks axon_hooks; provide it so trace=True can
    drive NTFF profiling through libaxon_pjrt's C ABI."""
    import sys, types
    if "antenv.axon_hooks" in sys.modules:
        return
    mod = types.ModuleType("antenv.axon_hooks")
    holder = [None]
    mod.set_axon_ntff_profile_hook = lambda h: holder.__setitem__(0, h)
    mod.get_axon_ntff_profile_hook = lambda: holder[0]
    sys.modules["antenv.axon_hooks"] = mod
    try:
        from trn_agent_boot.trn_boot import _ntff_profile_via_ctypes
        holder[0] = _ntff_profile_via_ctypes("/opt/axon/libaxon_pjrt.so")
    except Exception:
        pass


_program = None

def _get_program():
    global _program
    if _program is None:
        _program = build_program()
    return _program


def _chunk_bias(b, nch):
    out = np.zeros((128, nch), np.float32)
    out[:, :] = np.asarray(b, np.float32).reshape(nch, 128).T
    return out


E4 = ml_dtypes.float8_e4m3
F16NP = np.float16


def kernel(**inputs):
    global _last_results
    fp = {k: np.asarray(v) for k, v in inputs.items()}
    tokens = np.asarray(fp["tokens"], np.float32)

    # --- screen construction: 126-unit subset + rank-1 linear compensation
    p_w1 = np.asarray(fp["p_w1"], np.float32)
    p_w2v = np.asarray(fp["p_w2"], np.float32)[:, 0]
    p_b1 = np.asarray(fp["p_b1"], np.float32)
    crit = np.abs(p_w2v) * np.linalg.norm(p_w1, axis=0)
    sub_order = np.argsort(-crit)
    nsub = HSUB - 2
    S = np.sort(sub_order[:nsub]); Sd = np.sort(sub_order[nsub:])
    W1s = p_w1[:, S]; w2s = p_w2v[S]; b1s = p_b1[S]
    u = 0.5 * (p_w1[:, Sd] * p_w2v[Sd]).sum(axis=1)
    un = np.linalg.norm(u)
    if un > 0:
        v = u / un * np.linalg.norm(W1s, axis=0).mean()
        wv = un / np.linalg.norm(v)
    else:
        v = np.zeros(C, np.float32); wv = 0.0
    W1x = np.concatenate([W1s, v[:, None], -v[:, None]], axis=1)
    b1x = np.concatenate([b1s, [0.0, 0.0]]).astype(np.float32)
    w2x = np.concatenate([w2s, [wv, -wv]]).astype(np.float32)

    w2sel = np.zeros((128, 256), np.float32)
    for i in range(16):
        w2sel[:, 16 * i + i] = w2x
    w2f64 = np.zeros((128, 4 * M4 * 4), np.float32)
    for m in range(M4):
        for sl in range(4):
            w2f64[:, 4 * (4 * m + sl) + sl] = p_w2v[m * 128:(m + 1) * 128]

    consts = np.zeros((128, 40), np.float32)
    consts[:, 0:1] = (b1x * 16.0).reshape(128, 1)
    consts[:, 4:8] = _chunk_bias(fp["enc_b"], M4)
    consts[:, 8:10] = _chunk_bias(fp["a_b1"], KH2)
    consts[:, 10:12] = _chunk_bias(fp["k_b1"], KH2)
    consts[:, 12:16] = _chunk_bias(fp["r_b1"], M4)
    consts[:, 16:24] = _chunk_bias(fp["r_b2"], KC)
    consts[:, 24:28] = _chunk_bias(fp["f_b1"], M4)
    consts[:, 28:36] = _chunk_bias(fp["f_b2"], KC)
    consts[:, 36:40] = _chunk_bias(p_b1, M4)

    # rowbase: score row r = 4*sl + q  -> global row (4b+sl)*N + q*QW
    rb = {}
    for b in range(2):
        arr = np.zeros((16, 1), np.float32)
        for r in range(16):
            sl, q = divmod(r, 4)
            arr[r, 0] = (4 * b + sl) * N + q * QW
        rb[b] = arr

    shared = dict(
        w1dr=(W1x * 16.0).astype(E4),
        w2sel=w2sel.astype(F16NP),
        pw1f=p_w1.astype(F16NP),
        w2f64=w2f64,
        enc_w=np.asarray(fp["enc_w"], F16NP),
        a_w1=np.asarray(fp["a_w1"], F16NP),
        a_w2=np.asarray(fp["a_w2"], F16NP),
        k_w1=np.asarray(fp["k_w1"], F16NP),
        k_w2=np.asarray(fp["k_w2"], F16NP),
        r_w1=np.asarray(fp["r_w1"], F16NP),
        r_w2=np.asarray(fp["r_w2"], F16NP),
        f_w1=np.asarray(fp["f_w1"], F16NP),
        f_w2=np.asarray(fp["f_w2"], F16NP),
        a_b2=np.asarray(fp["a_b2"], np.float32).reshape(1, 1),
        k_b2=np.asarray(fp["k_b2"], np.float32).reshape(1, 1),
        consts=consts,
        rowbase0=rb[0], rowbase1=rb[1],
    )

    in_maps = []
    for c in range(NCORES):
        sh = tokens[BS * c:BS * (c + 1)]                  # [BS, N, C]
        # packed transposed fp8: [128, BS, KC, N], contiguous per partition
        t8 = np.ascontiguousarray(
            sh.reshape(BS, N, KC, 128).transpose(3, 0, 2, 1)).astype(E4)
        m = dict(shared)
        m["tok_nat"] = sh.reshape(R, C)
        m["tok8p"] = t8.reshape(128, BS * KC * N)
        in_maps.append(m)

    nc = _get_program()
    trace = bool(os.environ.get("ATSA_TRACE"))
    if trace:
        _install_ntff_shim()
    res = run_bass_kernel_spmd(nc, in_maps, list(range(NCORES)), trace=trace)
    _last_results = res

    out = np.empty((B, C), np.float32)
    for c in range(NCORES):
        out[BS * c:BS * (c + 1)] = res.results[c]["out_t"].T
    return out


# revision 19
# speedup vs baseline: 1.0929x; 1.0929x over previous
"""Trainium2 Bass kernel for nn_ATSA_56384330662502 (topk_masking), v3.

Math (validated against the reference in fp-sim, rel err ~1.2e-3, tol 2e-2):
  a_k selection needs only the top-a_k tokens by imp (pooled_sum telescopes to
  total - sum_selected).  The screen therefore only has to put the true top
  token(s) into a candidate set; an exact fp16 rescore picks the winner.

  screen: 128-wide hidden subset of p_w1/p_w2 (126 units with largest
  |w2_h|*||W1_h|| + 2 slots (+v,-v) that synthesize the rank-1 linear
  correction 0.5*sum_dropped w2_h W1_h of the dropped units; relu(x)-relu(-x)
  = x).  fp8-e4m3 DoubleRow matmuls (weights *16).  Scores are produced per
  (sample, 512-token quarter) in a [16, 512] layout so ONE DVE max/max_index
  call yields top-8 per quarter = 32 candidates/sample (validated: true top-1
  has worst in-quarter screen rank 2).
  rescore: gather the 32 fp32 rows, exact fp16 full-H rescore ranks them
  (validated exact for every sample).
  totals: fp8 sums split across DVE (tensor_tensor_reduce, 2 elem/cyc),
  ACT (accum-copy) and GPSIMD (pair-add) so no engine is the bottleneck.
  pooled = (total - sum_sel)/(N - a_k); agg = (sum_ref + pooled)/(a_k+1)
  out = mlp2(agg, f_*)  (f16 weights)

Sharding: data-parallel over batch, 8 samples/core.  tok8 ships transposed
e4m3 packed so each sample's DMA is one contiguous 16KB descriptor per
partition; tok_nat fp32 natural is only touched by the candidate gather.
"""
import os
import numpy as np
import ml_dtypes

import concourse.bass as bass
import concourse.mybir as mybir
import concourse.bacc as bacc
import concourse.tile as tile
from concourse.bass_utils import run_bass_kernel_spmd
from concourse.masks import make_identity

F32 = mybir.dt.float32
F16 = mybir.dt.float16
FP8 = mybir.dt.float8e4
U32 = mybir.dt.uint32
I32 = mybir.dt.int32
AF = mybir.ActivationFunctionType
OP = mybir.AluOpType
AX = mybir.AxisListType
DR = mybir.MatmulPerfMode.DoubleRow

B, N, C, H = 64, 2048, 1024, 512
NCORES = 8
BS = B // NCORES            # 8 samples per core
R = BS * N                  # 16384 token rows per core
KC = C // 128               # 8 contraction chunks
JP = KC // 2                # 4 DoubleRow chunk-pairs
M4 = H // 128               # 4 chunks of H
NQ = 4                      # quarters per sample
QW = N // NQ                # 512
HSUB = 128                  # screen hidden width (126 subset + 2 comp)
KD = 8                      # top-8 per quarter
CPS = NQ * KD               # 32 candidates per sample
CPB = 4 * CPS               # 128 candidates per 4-sample batch
H2 = H // 2                 # 256
KH2 = H2 // 128             # 2

# totals chunk split: chunks 0..NDV-1 -> DVE plain reduce, next NAC -> ACT
# accum-copy, rest -> GPSIMD pair-add + DVE fp16 finish
NDV = 2
NAC = 3

_last_results = None


def _floor_pos(nc, pool, src_ap, tag):
    """floor(x) for x >= 0; fp32->int32 cast is round-to-nearest-even, so
    floor(x) == rne(x - 0.5) (x never an exact integer here)."""
    ti = pool.tile([1, BS], I32, tag=tag + "_i", name=tag + "_i")
    tf = pool.tile([1, BS], F32, tag=tag + "_f", name=tag + "_f")
    th = pool.tile([1, BS], F32, tag=tag + "_h", name=tag + "_h")
    nc.vector.tensor_scalar(th[:], src_ap, 0.5, None, op0=OP.subtract)
    nc.vector.tensor_copy(ti[:], th[:])
    nc.vector.tensor_copy(tf[:], ti[:])
    return tf


def build_program():
    nc = bacc.Bacc("TRN2", target_bir_lowering=False, debug=False,
                   num_devices=NCORES)

    def din(name, shape, dt=F32):
        return nc.dram_tensor(name, list(shape), dt, kind="ExternalInput").ap()

    tok8p = din("tok8p", [128, BS * KC * N], FP8)   # packed transposed shard
    tok_nat = din("tok_nat", [R, C])                # natural shard (gather)
    w1dr = din("w1dr", [C, HSUB], FP8)              # screen W1' * 16, e4m3
    w2sel = din("w2sel", [128, 256], F16)           # screen w2' sl-masked
    pw1f = din("pw1f", [C, H], F16)                 # p_w1 (rescore)
    w2f64 = din("w2f64", [128, 4 * M4 * 4])         # p_w2 sl-masked (rescore)
    enc_w = din("enc_w", [C, H], F16)
    a_w1 = din("a_w1", [H, H2], F16); a_w2 = din("a_w2", [H2, 1], F16)
    k_w1 = din("k_w1", [H, H2], F16); k_w2 = din("k_w2", [H2, 1], F16)
    r_w1 = din("r_w1", [C, H], F16); r_w2 = din("r_w2", [H, C], F16)
    f_w1 = din("f_w1", [C, H], F16); f_w2 = din("f_w2", [H, C], F16)
    a_b2 = din("a_b2", [1, 1]); k_b2 = din("k_b2", [1, 1])
    thr20 = din("thr20", [1, 20])
    consts = din("consts", [128, 40])               # bundled biases
    rowbase = {b: din(f"rowbase{b}", [16, 1]) for b in range(2)}

    out_t = nc.dram_tensor("out_t", [C, BS], F32, kind="ExternalOutput").ap()

    with tile.TileContext(nc) as tc:
        with tc.tile_pool(name="wp", bufs=1) as wp, \
             tc.tile_pool(name="xb", bufs=3) as xbp, \
             tc.tile_pool(name="rh", bufs=2) as rhp, \
             tc.tile_pool(name="jk", bufs=2) as jkp, \
             tc.tile_pool(name="gb", bufs=3) as gbp, \
             tc.tile_pool(name="sc", bufs=2) as scp, \
             tc.tile_pool(name="ps", bufs=2, space="PSUM") as php, \
             tc.tile_pool(name="pt", bufs=2, space="PSUM") as ptp, \
             tc.tile_pool(name="pi", bufs=1, space="PSUM") as pip, \
             tc.tile_pool(name="dp", bufs=1, space="DRAM") as dp:

            # ---- persistent fp8 weights + consts (needed before sample 0) ----
            w1sb = wp.tile([128, KC, HSUB], FP8, tag="w1sb", name="w1sb")
            nc.sync.dma_start(w1sb[:], w1dr.rearrange("(j p) h -> p j h", p=128))
            w2sb = wp.tile([128, 256], F16, tag="w2sb", name="w2sb")
            nc.sync.dma_start(w2sb[:], w2sel)
            cst = wp.tile([128, 40], F32, tag="cst", name="cst")
            nc.sync.dma_start(cst[:], consts)
            pb1s = cst[:, 0:1]    # screen relu bias (*16)
            encb = cst[:, 4:8]; ab1 = cst[:, 8:10]; kb1 = cst[:, 10:12]
            rb1 = cst[:, 12:16]; rb2 = cst[:, 16:24]
            fb1 = cst[:, 24:28]; fb2 = cst[:, 28:36]
            pb1 = cst[:, 36:40]   # p_b1 (rescore relu)
            rwb = {}
            for b in range(2):
                rwb[b] = wp.tile([16, 1], F32, tag=f"rwb{b}", name=f"rwb{b}")
                nc.sync.dma_start(rwb[b][:], rowbase[b])

            tot3 = wp.tile([128, KC, BS], F32, tag="tot3", name="tot3")
            impq = {b: wp.tile([16, QW], F32, tag=f"impq{b}", name=f"impq{b}")
                    for b in range(2)}
            mx = {b: wp.tile([16, 8], F32, tag=f"mx{b}", name=f"mx{b}")
                  for b in range(2)}
            ixf = {b: wp.tile([16, KD], U32, tag=f"ixf{b}", name=f"ixf{b}")
                   for b in range(2)}
            ixg = {b: wp.tile([16, KD], F32, tag=f"ixg{b}", name=f"ixg{b}")
                   for b in range(2)}
            ixi = {b: wp.tile([16, KD], I32, tag=f"ixi{b}", name=f"ixi{b}")
                   for b in range(2)}
            gidx = {b: wp.tile([CPB, 1], I32, tag=f"gidx{b}", name=f"gidx{b}")
                    for b in range(2)}
            gath = {b: wp.tile([CPB, C], F32, tag=f"gath{b}", name=f"gath{b}")
                    for b in range(2)}
            gathT = {b: wp.tile([128, KC * CPB], F32, tag=f"gT{b}",
                                name=f"gT{b}") for b in range(2)}
            gathT16 = {b: wp.tile([128, KC * CPB], F16, tag=f"gT16{b}",
                                  name=f"gT16{b}") for b in range(2)}
            rhr = {b: wp.tile([128, M4 * CPB], F32, tag=f"rhr{b}",
                              name=f"rhr{b}") for b in range(2)}
            impr = {b: wp.tile([4, CPS], F32, tag=f"impr{b}", name=f"impr{b}")
                    for b in range(2)}
            akf4 = {b: wp.tile([4, 1], F32, tag=f"akf4{b}", name=f"akf4{b}")
                    for b in range(2)}
            mflat = {b: wp.tile([1, CPB], F32, tag=f"mf{b}", name=f"mf{b}")
                     for b in range(2)}
            scrm = dp.tile([CPB, 2], F32, tag="scrm", name="scrm")
            bcm = {b: wp.tile([128, CPB], F32, tag=f"bcm{b}", name=f"bcm{b}")
                   for b in range(2)}
            rr = {b: wp.tile([128, M4 * CPB], F16, tag=f"rr{b}",
                             name=f"rr{b}") for b in range(2)}
            rrs = {b: wp.tile([128, M4 * 4], F16, tag=f"rrs{b}",
                              name=f"rrs{b}") for b in range(2)}
            rrsF = wp.tile([128, M4 * 4], F32, tag="rrsF", name="rrsF")
            srefA = wp.tile([128, KC * BS], F32, tag="srefA", name="srefA")
            sselA = wp.tile([128, KC * BS], F32, tag="sselA", name="sselA")
            scratch = dp.tile([CPB, 2], I32, tag="scratch", name="scratch")

            W = {}

            def load_mat(dram, kdim, mwidth, dt, name):
                t = wp.tile([128, kdim * mwidth], dt, tag=name, name=name)
                nc.sync.dma_start(
                    t[:].rearrange("p (k m) -> p k m", k=kdim),
                    dram.rearrange("(k p) m -> p k m", p=128))
                W[name] = t
                return t

            def tail_weights_a():
                load_mat(enc_w, KC, H, F16, "encw")
                load_mat(a_w1, M4, H2, F16, "aw1")
                load_mat(a_w2, KH2, 1, F16, "aw2")
                load_mat(k_w1, M4, H2, F16, "kw1")
                load_mat(k_w2, KH2, 1, F16, "kw2")
                w2fsb = wp.tile([128, 4 * M4 * 4], F32, tag="w2fsb",
                                name="w2fsb")
                nc.sync.dma_start(w2fsb[:], w2f64)
                W["w2fsb"] = w2fsb
                ab2t = wp.tile([1, 1], F32, tag="ab2", name="ab2")
                nc.sync.dma_start(ab2t[:], a_b2)
                W["nab2"] = ab2t
                kb2t = wp.tile([1, 1], F32, tag="kb2", name="kb2")
                nc.sync.dma_start(kb2t[:], k_b2)
                W["kb2"] = kb2t
                tht = wp.tile([1, 20], F32, tag="thr", name="thr")
                nc.sync.dma_start(tht[:], thr20)
                W["thr"] = tht
                ident = wp.tile([128, 128], F32, tag="ident", name="ident")
                make_identity(nc, ident[:])
                W["ident"] = ident
                ones1 = wp.tile([1, 128], F32, tag="ones1", name="ones1")
                nc.gpsimd.memset(ones1[:], 1.0)
                W["ones1"] = ones1

            # ============== tail stages (per 4-sample batch) ==============
            def top8_batch(b):
                nc.scalar.activation(impq[b][:], SC[b][:], AF.Copy)
                nc.vector.max(mx[b][:], impq[b][:])
                nc.vector.max_index(ixf[b][:], mx[b][:], impq[b][:])
                nc.vector.tensor_copy(ixg[b][:], ixf[b][:])
                nc.vector.tensor_scalar(ixg[b][:], ixg[b][:],
                                        rwb[b][:], None, op0=OP.add)
                nc.vector.tensor_copy(ixi[b][:], ixg[b][:])
                # bounce [16, 8] -> [128, 1] through a DRAM tile (dep-tracked)
                nc.sync.dma_start(
                    scratch[:, b:b + 1]
                    .rearrange("(r c) x -> r (c x)", c=KD),
                    ixi[b][:])
                nc.sync.dma_start(gidx[b][:], scratch[:, b:b + 1])
                nc.gpsimd.indirect_dma_start(
                    out=gath[b][:], out_offset=None, in_=tok_nat,
                    in_offset=bass.IndirectOffsetOnAxis(ap=gidx[b][:, 0:1],
                                                        axis=0))

            def transpose_batch(b, ident):
                # gath [CPB, C] fp32 -> gathT [128, (cc, cand)] + f16 copy
                for g in range(2):          # two groups of 4 chunks
                    pt = ptp.tile([128, 512], F32, tag="ph", name="pt")
                    for cc in range(4 * g, 4 * g + 4):
                        nc.tensor.transpose(
                            pt[:, 128 * (cc - 4 * g):128 * (cc - 4 * g) + 128],
                            gath[b][:, 128 * cc:128 * (cc + 1)], ident[:])
                    lo = 512 * g
                    nc.scalar.activation(gathT[b][:, lo:lo + 512], pt[:],
                                         AF.Copy)
                    nc.vector.tensor_copy(gathT16[b][:, lo:lo + 512],
                                          gathT[b][:, lo:lo + 512])

            def rescore_batch(b, pw1sb, w2fsb, ones1, bcv, ident):
                # exact fp16 full-H rescore of the CPB gathered candidates
                pr = ptp.tile([128, M4 * 128], F32, tag="ph", name="pr")
                for m in range(M4):
                    for j in range(KC):
                        nc.tensor.matmul(
                            pr[:, 128 * m:128 * (m + 1)],
                            pw1sb[:, H * j + 128 * m:H * j + 128 * (m + 1)],
                            gathT16[b][:, 128 * j:128 * (j + 1)],
                            start=(j == 0), stop=(j == KC - 1))
                for m in range(M4):
                    nc.scalar.activation(
                        rhr[b][:, 128 * m:128 * (m + 1)],
                        pr[:, 128 * m:128 * (m + 1)],
                        AF.Relu, bias=pb1[:, m:m + 1])
                # stage 2 into [4 samples, CPS] layout via sl-masked w2
                pR = ptp.tile([4, CPS], F32, tag="ph", name="pR")
                for m in range(M4):
                    for sl in range(4):
                        nc.tensor.matmul(
                            pR[:], w2fsb[:, 4 * (4 * m + sl):
                                         4 * (4 * m + sl) + 4],
                            rhr[b][:, 128 * m + CPS * sl:
                                   128 * m + CPS * sl + CPS],
                            start=(m == 0 and sl == 0),
                            stop=(m == M4 - 1 and sl == 3))
                nc.scalar.activation(impr[b][:], pR[:], AF.Copy)
                # rank candidates within their sample, mask = rank < a_k
                cmp4 = scp.tile([4, CPS * CPS], F32, tag="cmp4",
                                name="cmp4", bufs=1)
                vA = impr[b][:].rearrange("p (c o) -> p c o", o=1) \
                    .to_broadcast([4, CPS, CPS])
                vB = impr[b][:].rearrange("p (o c) -> p o c", o=1) \
                    .to_broadcast([4, CPS, CPS])
                nc.vector.tensor_tensor(
                    cmp4[:].rearrange("p (c o) -> p c o", o=CPS),
                    vB, vA, op=OP.is_gt)
                rank = scp.tile([4, CPS], F32, tag="rank", name="rank",
                                bufs=1)
                nc.vector.tensor_reduce(
                    rank[:], cmp4[:].rearrange("p (c o) -> p c o", o=CPS),
                    axis=AX.X, op=OP.add)
                # a_k per sample as a [4, 1] column via PE transpose
                pak = ptp.tile([4, 1], F32, tag="ph", name="pak")
                nc.tensor.transpose(pak[:],
                                    bcv[:, 2 * BS + 4 * b:2 * BS + 4 * b + 4],
                                    ident[0:1, 0:1])
                nc.scalar.activation(akf4[b][:], pak[:], AF.Copy)
                mask1 = scp.tile([4, CPS], F32, tag="mask1", name="mask1",
                                 bufs=1)
                nc.vector.tensor_scalar(mask1[:], rank[:], akf4[b][:, 0:1],
                                        None, op0=OP.is_lt)
                # bounce [4, CPS] -> [1, CPB], broadcast to 128 rows via PE
                nc.sync.dma_start(
                    scrm[:, b:b + 1].rearrange("(s c) x -> s (c x)", c=CPS),
                    mask1[:])
                nc.sync.dma_start(
                    mflat[b][:],
                    scrm[:, b:b + 1].rearrange("(a c) x -> a (c x)", c=CPB))
                pbm = ptp.tile([128, CPB], F32, tag="ph", name="pbm")
                nc.tensor.matmul(pbm[:], ones1[:], mflat[b][:], start=True,
                                 stop=True)
                nc.scalar.activation(bcm[b][:], pbm[:], AF.Copy)

            def refine_batch(b, rw1, rw2):
                # mlp2(cand, r_*) for all CPB candidates, mask-summed / sample
                prf = ptp.tile([128, M4 * 128], F32, tag="ph", name="prf")
                for m in range(M4):
                    for j in range(KC):
                        nc.tensor.matmul(
                            prf[:, 128 * m:128 * (m + 1)],
                            rw1[:, H * j + 128 * m:H * j + 128 * (m + 1)],
                            gathT16[b][:, 128 * j:128 * (j + 1)],
                            start=(j == 0), stop=(j == KC - 1))
                for m in range(M4):
                    nc.scalar.activation(
                        rr[b][:, 128 * m:128 * (m + 1)],
                        prf[:, 128 * m:128 * (m + 1)],
                        AF.Relu, bias=rb1[:, m:m + 1])
                # mask + per-sample presum over candidates (32 contiguous)
                rrm = scp.tile([128, M4 * CPB], F16, tag="rrm", name="rrm",
                               bufs=1)
                nc.vector.tensor_tensor(
                    rrm[:].rearrange("p (m c) -> p m c", m=M4),
                    rr[b][:].rearrange("p (m c) -> p m c", m=M4),
                    bcm[b][:].rearrange("p (o c) -> p o c", o=1)
                    .to_broadcast([128, M4, CPB]), op=OP.mult)
                nc.vector.tensor_reduce(
                    rrsF[:],
                    rrm[:].rearrange("p (ms c) -> p ms c", c=CPS),
                    axis=AX.X, op=OP.add)
                nc.vector.tensor_copy(rrs[b][:], rrsF[:])
                # stage 2: out[c-chunk, sample] directly
                prg = ptp.tile([128, KC * 4], F32, tag="ph", name="prg")
                for cc in range(KC):
                    for m in range(M4):
                        nc.tensor.matmul(
                            prg[:, 4 * cc:4 * cc + 4],
                            rw2[:, C * m + 128 * cc:C * m + 128 * (cc + 1)],
                            rrs[b][:, 4 * m:4 * m + 4],
                            start=(m == 0), stop=(m == M4 - 1))
                nc.scalar.activation(
                    srefA[:].rearrange("p (k s) -> p k s", k=KC)
                    [:, :, 4 * b:4 * b + 4],
                    prg[:].rearrange("p (k s) -> p k s", k=KC), AF.Copy)

            def selsum_batch(b):
                # sum of selected raw rows per sample (fp32, from gathT)
                selm = scp.tile([128, KC * CPB], F32, tag="selm", name="selm",
                                bufs=1)
                nc.vector.tensor_tensor(
                    selm[:].rearrange("p (k c) -> p k c", k=KC),
                    gathT[b][:].rearrange("p (k c) -> p k c", k=KC),
                    bcm[b][:].rearrange("p (o c) -> p o c", o=1)
                    .to_broadcast([128, KC, CPB]), op=OP.mult)
                nc.vector.tensor_reduce(
                    sselA[:].rearrange("p (k s) -> p k s", k=KC)
                    [:, :, 4 * b:4 * b + 4],
                    selm[:].rearrange("p (k s c) -> p k s c", k=KC, c=CPS),
                    axis=AX.X, op=OP.add)

            # ---- router (per 4-sample batch, needs tot3 cols done) ----
            def router_batch(b, encw, aw1, aw2, kw1, kw2, ab2t, kb2t,
                             featT16, bcv, meanT16):
                thrt = W["thr"]
                s0, s1 = 4 * b, 4 * b + 4
                nc.vector.tensor_scalar(
                    meanT16[:, :, s0:s1], tot3[:, :, s0:s1], 1.0 / N, None,
                    op0=OP.mult)
                for m in range(M4):
                    pf = ptp.tile([128, 4], F32, tag="ph", name="pf")
                    for j in range(KC):
                        nc.tensor.matmul(pf[:], encw[:, H * j + 128 * m:
                                                     H * j + 128 * (m + 1)],
                                         meanT16[:, j, s0:s1], start=(j == 0),
                                         stop=(j == KC - 1))
                    nc.scalar.activation(featT16[m][:, s0:s1], pf[:], AF.Relu,
                                         bias=encb[:, m:m + 1])

                def head(w1t, b1c, w2t, b2t, kind, o):
                    # exp-only: sigmoid = 1/(1+exp(-x)); k via threshold count
                    h1 = {}
                    for mh in range(KH2):
                        p1 = ptp.tile([128, 4], F32, tag="ph", name="p1")
                        for k in range(M4):
                            nc.tensor.matmul(
                                p1[:], w1t[:, H2 * k + 128 * mh:
                                           H2 * k + 128 * (mh + 1)],
                                featT16[k][:, s0:s1], start=(k == 0),
                                stop=(k == M4 - 1))
                        t1 = scp.tile([128, 4], F16, tag=f"hh{o}{mh}{b}",
                                      name=f"hh{o}{mh}{b}", bufs=1)
                        nc.scalar.activation(t1[:], p1[:], AF.Relu,
                                             bias=b1c[:, mh:mh + 1])
                        h1[mh] = t1
                    p2 = ptp.tile([1, 4], F32, tag="ph", name=f"p2{o}")
                    for k in range(KH2):
                        nc.tensor.matmul(p2[:], w2t[:, k:k + 1], h1[k][:],
                                         start=(k == 0), stop=(k == KH2 - 1))
                    e = scp.tile([1, BS], F32, tag=f"se{o}", name=f"se{o}",
                                 bufs=1)
                    if kind == "sigmoid":
                        # e = exp(-(x + b2));  alpha = 1/(1+e)
                        nc.scalar.activation(e[:, s0:s1], p2[:], AF.Exp,
                                             bias=b2t[:], scale=-1.0)
                        nc.vector.tensor_scalar(e[:, s0:s1], e[:, s0:s1], 1.0,
                                                None, op0=OP.add)
                        nc.vector.reciprocal(alpha[:, s0:s1], e[:, s0:s1])
                    else:
                        # e = exp(x + b2); k = max(1, #{j: e >= e^(j-.5)-1})
                        nc.scalar.activation(e[:, s0:s1], p2[:], AF.Exp,
                                             bias=b2t[:])
                        cmp20 = scp.tile([1, 4 * 20], F32, tag=f"c20{b}",
                                         name=f"c20{b}", bufs=1)
                        vE = e[:, s0:s1].rearrange("p (s o) -> p s o", o=1) \
                            .to_broadcast([1, 4, 20])
                        vT = thrt[:].rearrange("p (o t) -> p o t", o=1) \
                            .to_broadcast([1, 4, 20])
                        nc.vector.tensor_tensor(
                            cmp20[:].rearrange("p (s t) -> p s t", s=4),
                            vE, vT, op=OP.is_ge)
                        nc.vector.tensor_reduce(
                            kf[:, s0:s1],
                            cmp20[:].rearrange("p (s t) -> p s t", s=4),
                            axis=AX.X, op=OP.add)
                        nc.vector.tensor_scalar_max(kf[:, s0:s1],
                                                    kf[:, s0:s1], 1.0)
                head(aw1, ab1, aw2, ab2t, "sigmoid", "a")
                head(kw1, kb1, kw2, kb2t, "count", "k")
                ak0 = scp.tile([1, BS], F32, tag="ak0", name="ak0", bufs=1)
                nc.vector.tensor_tensor(ak0[:, s0:s1], alpha[:, s0:s1],
                                        kf[:, s0:s1], op=OP.mult)
                ti2 = scp.tile([1, BS], I32, tag="aki", name="aki", bufs=1)
                tf2 = scp.tile([1, BS], F32, tag="akh", name="akh", bufs=1)
                nc.vector.tensor_scalar(tf2[:, s0:s1], ak0[:, s0:s1], 0.5,
                                        None, op0=OP.subtract)
                nc.vector.tensor_copy(ti2[:, s0:s1], tf2[:, s0:s1])
                nc.vector.tensor_copy(akf[:, s0:s1], ti2[:, s0:s1])
                nc.vector.tensor_scalar_max(akf[:, s0:s1], akf[:, s0:s1], 1.0)
                nc.vector.tensor_tensor(akf[:, s0:s1], akf[:, s0:s1],
                                        kf[:, s0:s1], op=OP.min)
                # bcv: [inv1 | inv2 | a_k]
                cnt = scp.tile([1, BS], F32, tag="cnt", name="cnt", bufs=1)
                nc.vector.tensor_scalar(cnt[:, s0:s1], akf[:, s0:s1], -1.0,
                                        float(N), op0=OP.mult, op1=OP.add)
                nc.vector.reciprocal(bcv[:, s0:s1], cnt[:, s0:s1])
                ak1 = scp.tile([1, BS], F32, tag="ak1", name="ak1", bufs=1)
                nc.vector.tensor_scalar(ak1[:, s0:s1], akf[:, s0:s1], 1.0,
                                        None, op0=OP.add)
                nc.vector.reciprocal(bcv[:, BS + s0:BS + s1], ak1[:, s0:s1])
                nc.vector.tensor_copy(bcv[:, 2 * BS + s0:2 * BS + s1],
                                      akf[:, s0:s1])

            # shared router tiles
            alpha = wp.tile([1, BS], F32, tag="alpha", name="alpha")
            kraw = wp.tile([1, BS], F32, tag="kraw", name="kraw")
            kf = wp.tile([1, BS], F32, tag="kf", name="kf")
            akf = wp.tile([1, BS], F32, tag="akf", name="akf")
            bcv = wp.tile([1, 3 * BS], F32, tag="bcv", name="bcv")
            meanT16 = wp.tile([128, KC, BS], F16, tag="meanT", name="meanT")
            featT16 = {m: wp.tile([128, BS], F16, tag=f"featT{m}",
                                  name=f"featT{m}") for m in range(M4)}
            SC = {}

            # ================= main fp8 stream =================
            pending = []

            def flush_s2():
                while pending:
                    fs, fh, frh = pending.pop(0)
                    fb, fsl = divmod(fs, 4)
                    if fb not in SC:
                        SC[fb] = pip.tile([16, QW], F32, tag="SC",
                                          name=f"SC{fb}")
                    for qh in range(2):
                        q = 2 * fh + qh
                        r = 4 * fsl + q
                        nc.tensor.matmul(
                            SC[fb][:], w2sb[:, 16 * r:16 * r + 16],
                            frh[:, q, :],
                            start=(r == 0), stop=(r == 15))

            for s in range(BS):
                b, sl = divmod(s, 4)
                xb = xbp.tile([128, KC, N], FP8, tag="xb", name="xb")
                nc.sync.dma_start(
                    xb[:].rearrange("p j n -> p (j n)"),
                    tok8p[:, KC * N * s:KC * N * (s + 1)])
                # screen: two halves of 2 quarters each; stage-2 trails by
                # one half so the PE never waits on the relu copy-out
                rh16 = rhp.tile([128, NQ, QW], F16, tag="rh16", name="rh16")
                for hh in range(2):
                    ps1 = php.tile([128, 1024], F32, tag="ps1", name="ps1")
                    for jp in range(JP):
                        for qh in range(2):
                            q = 2 * hh + qh
                            nc.tensor.matmul(
                                ps1[:, 512 * qh:512 * qh + 512],
                                w1sb[:, 2 * jp:2 * jp + 2, :],
                                xb[:, 2 * jp:2 * jp + 2, QW * q:QW * (q + 1)],
                                start=(jp == 0), stop=(jp == JP - 1),
                                perf_mode=DR)
                    flush_s2()
                    dst = rh16[:, 2 * hh:2 * hh + 2, :] \
                        .rearrange("p a n -> p (a n)")
                    if hh == 0:
                        nc.scalar.activation(dst, ps1[:], AF.Relu,
                                             bias=pb1s[:])
                    else:
                        nc.vector.tensor_scalar(dst, ps1[:], pb1s[:], 0.0,
                                                op0=OP.add, op1=OP.max)
                    pending.append((s, hh, rh16))
                    if s == 4 and hh == 0:
                        # batch-0 scores complete (its last stage-2 flushed
                        # above); copy out before SC[1] reuses the bank
                        top8_batch(0)
                # totals: DVE {0,1} / ACT {2,3,4} / GP {5,6,7} + one DVE L2
                for j in range(2):
                    nc.vector.tensor_reduce(
                        tot3[:, j, s:s + 1], xb[:, j, :],
                        axis=AX.X, op=OP.add)
                junkA = scp.tile([128, N], FP8, tag="junkA", name="junkA",
                                 bufs=1)
                for j in range(2, 5):
                    nc.scalar.activation(
                        junkA[:], xb[:, j, :],
                        AF.Copy,
                        accum_out=tot3[:, j, s:s + 1])
                gbf = gbp.tile([128, 3, 1024], F16, tag="gbf", name="gbf")
                for i, j in enumerate(range(5, KC)):
                    nc.gpsimd.tensor_tensor(
                        gbf[:, i, :], xb[:, j, 0:1024], xb[:, j, 1024:2048],
                        op=OP.add)
                nc.vector.tensor_reduce(
                    tot3[:, 5:KC, s:s + 1], gbf[:],
                    axis=AX.X, op=OP.add)

                if s == 1:
                    tail_weights_a()
                if s == 2:
                    load_mat(pw1f, KC, H, F16, "pw1sb")
                if s == 3:
                    load_mat(r_w1, KC, H, F16, "rw1")
                if s == 4:
                    router_batch(0, W["encw"], W["aw1"], W["aw2"], W["kw1"],
                                 W["kw2"], W["nab2"], W["kb2"],
                                 featT16, bcv, meanT16)
                if s == 5:
                    load_mat(r_w2, M4, C, F16, "rw2")
                    transpose_batch(0, W["ident"])
                if s == 6:
                    rescore_batch(0, W["pw1sb"], W["w2fsb"], W["ones1"], bcv,
                                  W["ident"])
                if s == 7:
                    refine_batch(0, W["rw1"], W["rw2"])
                    selsum_batch(0)
            flush_s2()

            # ================= tail =================
            top8_batch(1)
            load_mat(f_w1, KC, H, F16, "fw1")
            load_mat(f_w2, M4, C, F16, "fw2")
            router_batch(1, W["encw"], W["aw1"], W["aw2"], W["kw1"],
                         W["kw2"], W["nab2"], W["kb2"],
                         featT16, bcv, meanT16)
            transpose_batch(1, W["ident"])
            rescore_batch(1, W["pw1sb"], W["w2fsb"], W["ones1"], bcv,
                          W["ident"])
            refine_batch(1, W["rw1"], W["rw2"])
            selsum_batch(1)

            # srefA += r_b2 * a_k  (broadcast both)
            pbc = ptp.tile([128, 3 * BS], F32, tag="ph", name="pbc")
            nc.tensor.matmul(pbc[:], W["ones1"][:], bcv[:], start=True,
                             stop=True)
            bc = wp.tile([128, 3 * BS], F32, tag="bc", name="bc")
            nc.scalar.activation(bc[:], pbc[:], AF.Copy)
            rbt = scp.tile([128, KC * BS], F32, tag="rbt", name="rbt")
            nc.vector.tensor_tensor(
                rbt[:].rearrange("p (c s) -> p c s", c=KC),
                rb2[:].rearrange("p (c o) -> p c o", o=1)
                .to_broadcast([128, KC, BS]),
                bc[:, 2 * BS:3 * BS].rearrange("p (o s) -> p o s", o=1)
                .to_broadcast([128, KC, BS]), op=OP.mult)
            nc.vector.tensor_tensor(srefA[:], srefA[:], rbt[:], op=OP.add)

            # pooled + agg
            poodA = scp.tile([128, KC * BS], F32, tag="poodA", name="poodA")
            nc.vector.tensor_tensor(
                poodA[:], tot3[:].rearrange("p c s -> p (c s)"), sselA[:],
                op=OP.subtract)
            nc.vector.tensor_tensor(
                poodA[:].rearrange("p (c s) -> p c s", c=KC),
                poodA[:].rearrange("p (c s) -> p c s", c=KC),
                bc[:, 0:BS].rearrange("p (o s) -> p o s", o=1)
                .to_broadcast([128, KC, BS]), op=OP.mult)
            nc.vector.tensor_tensor(poodA[:], poodA[:], srefA[:], op=OP.add)
            aggA = wp.tile([128, KC * BS], F16, tag="aggA", name="aggA")
            nc.vector.tensor_tensor(
                aggA[:].rearrange("p (c s) -> p c s", c=KC),
                poodA[:].rearrange("p (c s) -> p c s", c=KC),
                bc[:, BS:2 * BS].rearrange("p (o s) -> p o s", o=1)
                .to_broadcast([128, KC, BS]), op=OP.mult)

            # ---- final MLP (f16) ----
            ff1 = {}
            fw1 = W["fw1"]; fw2 = W["fw2"]
            for m in range(M4):
                pf1 = ptp.tile([128, BS], F32, tag="ph", name="pf1")
                for k in range(KC):
                    nc.tensor.matmul(pf1[:], fw1[:, H * k + 128 * m:
                                                 H * k + 128 * (m + 1)],
                                     aggA[:, BS * k:BS * (k + 1)],
                                     start=(k == 0), stop=(k == KC - 1))
                t = scp.tile([128, BS], F16, tag=f"ff1_{m}", name=f"ff1_{m}",
                             bufs=1)
                nc.scalar.activation(t[:], pf1[:], AF.Relu, bias=fb1[:, m:m + 1])
                ff1[m] = t
            for cc in range(KC):
                po = ptp.tile([128, BS], F32, tag="ph", name="po")
                for m in range(M4):
                    nc.tensor.matmul(po[:], fw2[:, C * m + 128 * cc:
                                                C * m + 128 * (cc + 1)],
                                     ff1[m][:], start=(m == 0),
                                     stop=(m == M4 - 1))
                oc = scp.tile([128, BS], F32, tag="oc", name="oc")
                nc.vector.tensor_scalar(oc[:], po[:], fb2[:, cc:cc + 1], None,
                                        op0=OP.add)
                nc.sync.dma_start(out_t[128 * cc:128 * (cc + 1), :], oc[:])

    nc.compile()
    return nc


def _install_ntff_shim():
    """This image's antenv lacks axon_hooks; provide it so trace=True can
    drive NTFF profiling through libaxon_pjrt's C ABI."""
    import sys, types
    if "antenv.axon_hooks" in sys.modules:
        return
    mod = types.ModuleType("antenv.axon_hooks")
    holder = [None]
    mod.set_axon_ntff_profile_hook = lambda h: holder.__setitem__(0, h)
    mod.get_axon_ntff_profile_hook = lambda: holder[0]
    sys.modules["antenv.axon_hooks"] = mod
    try:
        from trn_agent_boot.trn_boot import _ntff_profile_via_ctypes
        holder[0] = _ntff_profile_via_ctypes("/opt/axon/libaxon_pjrt.so")
    except Exception:
        pass


_program = None

def _get_program():
    global _program
    if _program is None:
        _program = build_program()
    return _program


def _chunk_bias(b, nch):
    out = np.zeros((128, nch), np.float32)
    out[:, :] = np.asarray(b, np.float32).reshape(nch, 128).T
    return out


E4 = ml_dtypes.float8_e4m3
F16NP = np.float16


def kernel(**inputs):
    global _last_results
    fp = {k: np.asarray(v) for k, v in inputs.items()}
    tokens = np.asarray(fp["tokens"], np.float32)

    # --- screen construction: 126-unit subset + rank-1 linear compensation
    p_w1 = np.asarray(fp["p_w1"], np.float32)
    p_w2v = np.asarray(fp["p_w2"], np.float32)[:, 0]
    p_b1 = np.asarray(fp["p_b1"], np.float32)
    crit = np.abs(p_w2v) * np.linalg.norm(p_w1, axis=0)
    sub_order = np.argsort(-crit)
    nsub = HSUB - 2
    S = np.sort(sub_order[:nsub]); Sd = np.sort(sub_order[nsub:])
    W1s = p_w1[:, S]; w2s = p_w2v[S]; b1s = p_b1[S]
    u = 0.5 * (p_w1[:, Sd] * p_w2v[Sd]).sum(axis=1)
    un = np.linalg.norm(u)
    if un > 0:
        v = u / un * np.linalg.norm(W1s, axis=0).mean()
        wv = un / np.linalg.norm(v)
    else:
        v = np.zeros(C, np.float32); wv = 0.0
    W1x = np.concatenate([W1s, v[:, None], -v[:, None]], axis=1)
    b1x = np.concatenate([b1s, [0.0, 0.0]]).astype(np.float32)
    w2x = np.concatenate([w2s, [wv, -wv]]).astype(np.float32)

    w2sel = np.zeros((128, 256), np.float32)
    for i in range(16):
        w2sel[:, 16 * i + i] = w2x
    w2f64 = np.zeros((128, 4 * M4 * 4), np.float32)
    for m in range(M4):
        for sl in range(4):
            w2f64[:, 4 * (4 * m + sl) + sl] = p_w2v[m * 128:(m + 1) * 128]

    consts = np.zeros((128, 40), np.float32)
    consts[:, 0:1] = (b1x * 16.0).reshape(128, 1)
    consts[:, 4:8] = _chunk_bias(fp["enc_b"], M4)
    consts[:, 8:10] = _chunk_bias(fp["a_b1"], KH2)
    consts[:, 10:12] = _chunk_bias(fp["k_b1"], KH2)
    consts[:, 12:16] = _chunk_bias(fp["r_b1"], M4)
    consts[:, 16:24] = _chunk_bias(fp["r_b2"], KC)
    consts[:, 24:28] = _chunk_bias(fp["f_b1"], M4)
    consts[:, 28:36] = _chunk_bias(fp["f_b2"], KC)
    consts[:, 36:40] = _chunk_bias(p_b1, M4)

    # rowbase: score row r = 4*sl + q  -> global row (4b+sl)*N + q*QW
    rb = {}
    for b in range(2):
        arr = np.zeros((16, 1), np.float32)
        for r in range(16):
            sl, q = divmod(r, 4)
            arr[r, 0] = (4 * b + sl) * N + q * QW
        rb[b] = arr

    shared = dict(
        w1dr=(W1x * 16.0).astype(E4),
        w2sel=w2sel.astype(F16NP),
        pw1f=p_w1.astype(F16NP),
        w2f64=w2f64,
        enc_w=np.asarray(fp["enc_w"], F16NP),
        a_w1=np.asarray(fp["a_w1"], F16NP),
        a_w2=np.asarray(fp["a_w2"], F16NP),
        k_w1=np.asarray(fp["k_w1"], F16NP),
        k_w2=np.asarray(fp["k_w2"], F16NP),
        r_w1=np.asarray(fp["r_w1"], F16NP),
        r_w2=np.asarray(fp["r_w2"], F16NP),
        f_w1=np.asarray(fp["f_w1"], F16NP),
        f_w2=np.asarray(fp["f_w2"], F16NP),
        a_b2=-np.asarray(fp["a_b2"], np.float32).reshape(1, 1),
        k_b2=np.asarray(fp["k_b2"], np.float32).reshape(1, 1),
        consts=consts,
        rowbase0=rb[0], rowbase1=rb[1],
        thr20=(np.exp(np.arange(1, 21, dtype=np.float64) - 0.5) - 1.0)
        .astype(np.float32).reshape(1, 20),
    )

    in_maps = []
    for c in range(NCORES):
        sh = tokens[BS * c:BS * (c + 1)]                  # [BS, N, C]
        # packed transposed fp8: [128, BS, KC, N], contiguous per partition
        t8 = np.ascontiguousarray(
            sh.reshape(BS, N, KC, 128).transpose(3, 0, 2, 1)).astype(E4)
        m = dict(shared)
        m["tok_nat"] = sh.reshape(R, C)
        m["tok8p"] = t8.reshape(128, BS * KC * N)
        in_maps.append(m)

    nc = _get_program()
    trace = bool(os.environ.get("ATSA_TRACE"))
    if trace:
        _install_ntff_shim()
    res = run_bass_kernel_spmd(nc, in_maps, list(range(NCORES)), trace=trace)
    _last_results = res

    out = np.empty((B, C), np.float32)
    for c in range(NCORES):
        out[BS * c:BS * (c + 1)] = res.results[c]["out_t"].T
    return out
